# revision 10
# baseline (speedup 1.0000x reference)
"""DiffMamba layer on 8 Trainium2 NeuronCores.

Sharding: 8 cores = 4 samples x 2 mixers (SPMD program; per-core inputs pick
the mixer weights). Each core runs the full Mamba mixer for its (sample,
mixer) over the whole sequence, then cores exchange mixer outputs pairwise
(XLA ppermute) and each core of a pair computes the differential-combine +
layernorms for half of the sequence.

Layout: channels-on-partitions (d-major), tokens along the free dim.  The
selective scan uses the native tensor_tensor_scan (state = dA*state + dBx)
over 8 state planes, exploiting A[d,n] = -n  =>  dA_n = exp(-dt)^n.
The depthwise causal conv is folded into the in_proj matmul (K=4*384 with
token-shifted rhs slices).  norm1 scale/bias are folded into the in_proj
weights on the host.
"""

import numpy as np
import jax
import jax.numpy as jnp
from jax.sharding import Mesh, PartitionSpec as P, NamedSharding
from jax.experimental.shard_map import shard_map

import concourse.bass as bass
import concourse.tile as tile
from concourse import mybir
from concourse.bass2jax import bass_jit

F32 = mybir.dt.float32
AF = mybir.ActivationFunctionType
OP = mybir.AluOpType

DIM = 384
N = 8
DCONV = 4
DTR = 24
BB = 4
L = 3136
HALF = L // 2
LC = 392          # chunk length; 8 chunks; 4 per half
NCH = L // LC
NT = DIM // 128   # 3 channel tiles
EPS = 1e-5


def _bcast_ap(handle, parts):
    """AP reading handle[0,0,0] broadcast to (parts, 1) via partition-step 0."""
    a = handle[0, 0:1, 0:1]
    return bass.AP(tensor=a.tensor, offset=a.offset, ap=[[0, parts], [1, 1]])


# ---------------------------------------------------------------- mamba NEFF
@bass_jit
def _mamba_kernel(nc, x, fold_lhsT, bias4, wz_lhsT, bias_z, xprojT, dtprojT,
                  dt_bias, d_vec, outT, selbc_in, b4r0_in, b4r_in):
    """One (sample, mixer) Mamba pass.  x: (384, L) -> y: (384, L)."""
    y_out = nc.dram_tensor("y_out", [1, DIM, L], F32, kind="ExternalOutput")

    with tile.TileContext(nc) as tc:
        with (
            tc.tile_pool(name="persist", bufs=1) as persist,
            tc.tile_pool(name="rows", bufs=2) as rows,
            tc.tile_pool(name="io", bufs=2) as io,
            tc.tile_pool(name="work", bufs=2) as work,
            tc.tile_pool(name="scan", bufs=2) as scanp,
            tc.tile_pool(name="ps_mm", bufs=2, space="PSUM") as ps_mm,
            tc.tile_pool(name="ps_rep", bufs=2, space="PSUM") as ps_rep,
            tc.tile_pool(name="ps_row", bufs=2, space="PSUM") as ps_row,
            tc.tile_pool(name="ps_aux", bufs=2, space="PSUM") as ps_aux,
        ):
            # ---------------- persistent weights in SBUF
            w_fold = persist.tile([128, 4 * NT, DIM], F32, tag="w_fold", name="w_fold")
            nc.sync.dma_start(out=w_fold,
                              in_=fold_lhsT[0].rearrange("(a p) c -> p a c", p=128))
            w_b4 = persist.tile([4, DIM], F32, tag="w_b4", name="w_b4")
            nc.sync.dma_start(out=w_b4, in_=bias4[0])
            w_z = persist.tile([128, NT, DIM], F32, tag="w_z", name="w_z")
            nc.sync.dma_start(out=w_z, in_=wz_lhsT[0].rearrange("(a p) c -> p a c", p=128))
            w_bz = persist.tile([1, DIM], F32, tag="w_bz", name="w_bz")
            nc.sync.dma_start(out=w_bz, in_=bias_z[0])
            w_xp = persist.tile([128, NT, 40], F32, tag="w_xp", name="w_xp")
            nc.sync.dma_start(out=w_xp, in_=xprojT[0].rearrange("(a p) c -> p a c", p=128))
            w_dt = persist.tile([DTR, DIM], F32, tag="w_dt", name="w_dt")
            nc.sync.dma_start(out=w_dt, in_=dtprojT[0])
            w_out = persist.tile([128, NT, DIM], F32, tag="w_out", name="w_out")
            nc.sync.dma_start(out=w_out, in_=outT[0].rearrange("(a p) c -> p a c", p=128))
            c_dtb = persist.tile([128, NT], F32, tag="c_dtb", name="c_dtb")
            nc.sync.dma_start(out=c_dtb, in_=dt_bias[0].rearrange("(a p) o -> p (a o)", p=128))
            c_dv = persist.tile([128, NT], F32, tag="c_dv", name="c_dv")
            nc.sync.dma_start(out=c_dv, in_=d_vec[0].rearrange("(a p) o -> p (a o)", p=128))

            ones_row = persist.tile([1, LC], F32, tag="ones_row", name="ones_row")
            nc.vector.memset(ones_row, 1.0)
            eps_c = persist.tile([1, 1], F32, tag="eps_c", name="eps_c")
            nc.vector.memset(eps_c, EPS)
            ones_col = persist.tile([128, 1], F32, tag="ones_col", name="ones_col")
            nc.vector.memset(ones_col, 1.0)
            ones_1x128 = persist.tile([1, 128], F32, tag="ones_1x128", name="ones_1x128")
            nc.vector.memset(ones_1x128, 1.0)
            # row-broadcast selectors: slice i picks x_dbl row DTR+i (B rows
            # then C rows) and replicates it to all 128 partitions via K=40
            # matmul from base partition 0.  Constant pattern, host-provided.
            selbc = persist.tile([40, 2 * N * 128], F32, tag="selbc", name="selbc")
            nc.sync.dma_start(out=selbc, in_=selbc_in[0])

            # scan carry state (zeroed)
            carries = []
            for j in range(NT):
                cj = persist.tile([128, N], F32, tag=f"carry{j}", name=f"carry{j}")
                nc.vector.memset(cj, 0.0)
                carries.append(cj)

            # bias4 rhs rows: (4, LC); row0 ones; rows1-3: e_t on chunk 0
            b4_rhs0 = persist.tile([4, LC], F32, tag="b4_rhs0", name="b4_rhs0")
            nc.sync.dma_start(out=b4_rhs0, in_=b4r0_in[0])
            b4_rhs = persist.tile([4, LC], F32, tag="b4_rhs", name="b4_rhs")
            nc.sync.dma_start(out=b4_rhs, in_=b4r_in[0])

            for k in range(NCH):
                t0 = k * LC
                LH = LC + 3  # with left halo
                # ------- load x chunk with 3-col left halo
                xt = [io.tile([128, LH], F32, tag=f"xt{j}", name=f"xt{j}") for j in range(NT)]
                for j in range(NT):
                    if k == 0:
                        nc.vector.memset(xt[j][:, 0:3], 0.0)
                        nc.sync.dma_start(out=xt[j][:, 3:LH],
                                          in_=x[0, 128 * j : 128 * (j + 1), t0 : t0 + LC])
                    else:
                        nc.sync.dma_start(out=xt[j],
                                          in_=x[0, 128 * j : 128 * (j + 1), t0 - 3 : t0 + LC])

                # ------- norm1 stats over channels (PE ones-reduction)
                xsq = [work.tile([128, LH], F32, tag=f"xsq{j}", name=f"xsq{j}") for j in range(NT)]
                for j in range(NT):
                    nc.scalar.activation(out=xsq[j], in_=xt[j], func=AF.Square)
                ps_s = ps_row.tile([1, LH], F32, tag="lnrow", name="lnrow")
                ps_ss = ps_row.tile([1, LH], F32, tag="lnrow", name="lnrow")
                for j in range(NT):
                    nc.tensor.matmul(ps_s, lhsT=ones_col, rhs=xt[j],
                                     start=(j == 0), stop=(j == NT - 1))
                for j in range(NT):
                    nc.tensor.matmul(ps_ss, lhsT=ones_col, rhs=xsq[j],
                                     start=(j == 0), stop=(j == NT - 1))
                mean_r = rows.tile([1, LH], F32, tag="mean_r", name="mean_r")
                nc.scalar.mul(out=mean_r, in_=ps_s, mul=1.0 / DIM)
                ms_r = rows.tile([1, LH], F32, tag="ms_r", name="ms_r")
                nc.scalar.mul(out=ms_r, in_=ps_ss, mul=1.0 / DIM)
                var_r = rows.tile([1, LH], F32, tag="var_r", name="var_r")
                nc.vector.tensor_mul(out=var_r, in0=mean_r, in1=mean_r)
                nc.vector.tensor_sub(out=var_r, in0=ms_r, in1=var_r)
                lnv_r = rows.tile([1, LH], F32, tag="lnv_r", name="lnv_r")
                nc.scalar.activation(out=lnv_r, in_=var_r, func=AF.Ln, bias=eps_c)
                istd_r = rows.tile([1, LH], F32, tag="istd_r", name="istd_r")
                nc.scalar.activation(out=istd_r, in_=lnv_r, func=AF.Exp, scale=-0.5)
                mr_r = rows.tile([1, LH], F32, tag="mr_r", name="mr_r")
                nc.vector.tensor_mul(out=mr_r, in0=mean_r, in1=istd_r)
                # broadcast rows to 128 partitions via K=1 matmul
                ps_istd = ps_rep.tile([128, LH], F32, tag="rep", name="rep")
                nc.tensor.matmul(ps_istd, lhsT=ones_1x128, rhs=istd_r)
                ps_mr = ps_rep.tile([128, LH], F32, tag="rep", name="rep")
                nc.tensor.matmul(ps_mr, lhsT=ones_1x128, rhs=mr_r)
                # xn = x*istd - m*istd   (norm1 w/b folded into weights)
                xn = [io.tile([128, LH], F32, tag=f"xn{j}", name=f"xn{j}") for j in range(NT)]
                for j in range(NT):
                    nc.vector.tensor_mul(out=xn[j], in0=xt[j], in1=ps_istd)
                    nc.vector.tensor_sub(out=xn[j], in0=xn[j], in1=ps_mr)
                if k == 0:
                    for j in range(NT):
                        nc.vector.memset(xn[j][:, 0:3], 0.0)

                # ------- in_proj (conv folded) -> silu -> xs_act, z_silu
                xs_act = [work.tile([128, LC], F32, tag=f"xsact{j}", name=f"xsact{j}") for j in range(NT)]
                z_silu = [work.tile([128, LC], F32, tag=f"zsilu{j}", name=f"zsilu{j}") for j in range(NT)]
                brhs = b4_rhs0 if k == 0 else b4_rhs
                for mt in range(NT):
                    pxz = ps_mm.tile([128, LC], F32, tag="mm", name="mm")
                    first = True
                    for kk in range(DCONV):
                        for j in range(NT):
                            nc.tensor.matmul(
                                pxz,
                                lhsT=w_fold[:, kk * NT + j, 128 * mt : 128 * (mt + 1)],
                                rhs=xn[j][:, kk : kk + LC],
                                start=first, stop=False)
                            first = False
                    nc.tensor.matmul(pxz, lhsT=w_b4[:, 128 * mt : 128 * (mt + 1)],
                                     rhs=brhs, start=False, stop=True)
                    nc.scalar.activation(out=xs_act[mt], in_=pxz, func=AF.Silu)
                for mt in range(NT):
                    pz = ps_mm.tile([128, LC], F32, tag="mm", name="mm")
                    for j in range(NT):
                        nc.tensor.matmul(pz, lhsT=w_z[:, j, 128 * mt : 128 * (mt + 1)],
                                         rhs=xn[j][:, 3 : 3 + LC],
                                         start=(j == 0), stop=False)
                    nc.tensor.matmul(pz, lhsT=w_bz[:, 128 * mt : 128 * (mt + 1)],
                                     rhs=ones_row, start=False, stop=True)
                    nc.scalar.activation(out=z_silu[mt], in_=pz, func=AF.Silu)

                # ------- x_proj -> x_dbl (40, LC) in SBUF
                p_xd = ps_aux.tile([40, LC], F32, tag="aux", name="aux")
                for j in range(NT):
                    nc.tensor.matmul(p_xd, lhsT=w_xp[:, j, :], rhs=xs_act[j],
                                     start=(j == 0), stop=(j == NT - 1))
                xd = rows.tile([40, LC], F32, tag="xd", name="xd")
                nc.scalar.copy(out=xd, in_=p_xd)

                # ------- dt = softplus(dt_proj @ dt_raw + bias)
                # softplus(u) = ln(1 + e^u)  (no Softplus table in this build)
                dt_sb = [work.tile([128, LC], F32, tag=f"dt{j}", name=f"dt{j}") for j in range(NT)]
                for mt in range(NT):
                    pdt = ps_aux.tile([128, LC], F32, tag="aux", name="aux")
                    nc.tensor.matmul(pdt, lhsT=w_dt[:, 128 * mt : 128 * (mt + 1)],
                                     rhs=xd[0:DTR, :], start=True, stop=True)
                    eu = work.tile([128, LC], F32, tag="eu", name="eu")
                    nc.scalar.activation(out=eu, in_=pdt, func=AF.Exp,
                                         bias=c_dtb[:, mt : mt + 1])
                    nc.scalar.activation(out=dt_sb[mt], in_=eu, func=AF.Ln, bias=1.0)

                # ------- E1 = exp(-dt);  g = dt * xs
                e1 = [work.tile([128, LC], F32, tag=f"e1{j}", name=f"e1{j}") for j in range(NT)]
                g_sb = [work.tile([128, LC], F32, tag=f"g{j}", name=f"g{j}") for j in range(NT)]
                for j in range(NT):
                    nc.scalar.activation(out=e1[j], in_=dt_sb[j], func=AF.Exp, scale=-1.0)
                    nc.vector.tensor_mul(out=g_sb[j], in0=dt_sb[j], in1=xs_act[j])

                # ------- scan planes n = 1..8
                y_sb = [scanp.tile([128, LC], F32, tag=f"y{j}", name=f"y{j}") for j in range(NT)]
                ecur = [scanp.tile([128, LC], F32, tag=f"ec{j}", name=f"ec{j}") for j in range(NT)]
                for n in range(1, N + 1):
                    for j in range(NT):
                        if n == 2:
                            nc.vector.tensor_mul(out=ecur[j], in0=e1[j], in1=e1[j])
                        elif n > 2:
                            nc.vector.tensor_mul(out=ecur[j], in0=ecur[j], in1=e1[j])
                    dA = e1 if n == 1 else ecur
                    ps_bn = ps_rep.tile([128, LC], F32, tag="rep", name="rep")
                    nc.tensor.matmul(ps_bn, lhsT=selbc[:, 128 * (n - 1) : 128 * n],
                                     rhs=xd[0:40, :])
                    ps_cn = ps_rep.tile([128, LC], F32, tag="rep", name="rep")
                    nc.tensor.matmul(ps_cn, lhsT=selbc[:, 128 * (N + n - 1) : 128 * (N + n)],
                                     rhs=xd[0:40, :])
                    for j in range(NT):
                        dbx = scanp.tile([128, LC], F32, tag=f"dbx{j}", name=f"dbx{j}")
                        nc.vector.tensor_mul(out=dbx, in0=g_sb[j], in1=ps_bn)
                        h = scanp.tile([128, LC], F32, tag=f"h{j}", name=f"h{j}")
                        nc.vector.tensor_tensor_scan(
                            out=h, data0=dA[j], data1=dbx,
                            initial=carries[j][:, n - 1 : n],
                            op0=OP.mult, op1=OP.add)
                        nc.vector.tensor_copy(out=carries[j][:, n - 1 : n],
                                              in_=h[:, LC - 1 : LC])
                        if n == 1:
                            nc.vector.tensor_mul(out=y_sb[j], in0=h, in1=ps_cn)
                        else:
                            tmp = scanp.tile([128, LC], F32, tag=f"tmp{j}", name=f"tmp{j}")
                            nc.vector.tensor_mul(out=tmp, in0=h, in1=ps_cn)
                            nc.vector.tensor_add(out=y_sb[j], in0=y_sb[j], in1=tmp)

                # ------- y = (y + xs*D) * silu(z);  out_proj
                for j in range(NT):
                    nc.vector.scalar_tensor_tensor(
                        out=y_sb[j], in0=xs_act[j], scalar=c_dv[:, j : j + 1],
                        in1=y_sb[j], op0=OP.mult, op1=OP.add)
                    nc.vector.tensor_mul(out=y_sb[j], in0=y_sb[j], in1=z_silu[j])
                for mt in range(NT):
                    po = ps_mm.tile([128, LC], F32, tag="mm", name="mm")
                    for j in range(NT):
                        nc.tensor.matmul(po, lhsT=w_out[:, j, 128 * mt : 128 * (mt + 1)],
                                         rhs=y_sb[j], start=(j == 0), stop=(j == NT - 1))
                    o_sb = io.tile([128, LC], F32, tag=f"osb{mt}", name=f"osb{mt}")
                    nc.scalar.copy(out=o_sb, in_=po)
                    nc.sync.dma_start(out=y_out[0, 128 * mt : 128 * (mt + 1), t0 : t0 + LC],
                                      in_=o_sb)
    return y_out


# ------------------------------------------------------------- combine NEFF
@bass_jit
def _combine_kernel(nc, xh, y1h, y2h, neg_lam, w2b2, w3b3):
    """diff = y1 - lam*y2; attn = LN(diff)*w2+b2; out = LN(xh+attn)*w3+b3."""
    out = nc.dram_tensor("out_half", [1, DIM, HALF], F32, kind="ExternalOutput")

    with tile.TileContext(nc) as tc:
        with (
            tc.tile_pool(name="persist", bufs=1) as persist,
            tc.tile_pool(name="rows", bufs=2) as rows,
            tc.tile_pool(name="io", bufs=2) as io,
            tc.tile_pool(name="work", bufs=2) as work,
            tc.tile_pool(name="ps_row", bufs=2, space="PSUM") as ps_row,
            tc.tile_pool(name="ps_rep", bufs=2, space="PSUM") as ps_rep,
        ):
            eps_c = persist.tile([1, 1], F32, tag="eps_c", name="eps_c")
            nc.vector.memset(eps_c, EPS)
            ones_col = persist.tile([128, 1], F32, tag="ones_col", name="ones_col")
            nc.vector.memset(ones_col, 1.0)
            nl = persist.tile([128, 1], F32, tag="nl", name="nl")
            nc.sync.dma_start(out=nl, in_=_bcast_ap(neg_lam, 128))
            w2 = persist.tile([1, DIM], F32, tag="w2", name="w2")
            nc.sync.dma_start(out=w2, in_=w2b2[0, 0:1, :])
            b2 = persist.tile([128, NT], F32, tag="b2", name="b2")
            nc.sync.dma_start(out=b2,
                              in_=w2b2[0, 1:2, :].rearrange("o (a p) -> (o p) a", p=128))
            w3 = persist.tile([1, DIM], F32, tag="w3", name="w3")
            nc.sync.dma_start(out=w3, in_=w3b3[0, 0:1, :])
            b3 = persist.tile([128, NT], F32, tag="b3", name="b3")
            nc.sync.dma_start(out=b3,
                              in_=w3b3[0, 1:2, :].rearrange("o (a p) -> (o p) a", p=128))

            def layer_norm(v_tiles, w_row, b_col, out_tiles):
                vsq = [work.tile([128, LC], F32, tag=f"lnsq{j}", name=f"lnsq{j}") for j in range(NT)]
                for j in range(NT):
                    nc.scalar.activation(out=vsq[j], in_=v_tiles[j], func=AF.Square)
                ps_s = ps_row.tile([1, LC], F32, tag="lnrow", name="lnrow")
                ps_ss = ps_row.tile([1, LC], F32, tag="lnrow", name="lnrow")
                for j in range(NT):
                    nc.tensor.matmul(ps_s, lhsT=ones_col, rhs=v_tiles[j],
                                     start=(j == 0), stop=(j == NT - 1))
                for j in range(NT):
                    nc.tensor.matmul(ps_ss, lhsT=ones_col, rhs=vsq[j],
                                     start=(j == 0), stop=(j == NT - 1))
                mean_r = rows.tile([1, LC], F32, tag="lnmean", name="lnmean")
                nc.scalar.mul(out=mean_r, in_=ps_s, mul=1.0 / DIM)
                ms_r = rows.tile([1, LC], F32, tag="lnms", name="lnms")
                nc.scalar.mul(out=ms_r, in_=ps_ss, mul=1.0 / DIM)
                var_r = rows.tile([1, LC], F32, tag="lnvar", name="lnvar")
                nc.vector.tensor_mul(out=var_r, in0=mean_r, in1=mean_r)
                nc.vector.tensor_sub(out=var_r, in0=ms_r, in1=var_r)
                lnv_r = rows.tile([1, LC], F32, tag="lnlnv", name="lnlnv")
                nc.scalar.activation(out=lnv_r, in_=var_r, func=AF.Ln, bias=eps_c)
                istd_r = rows.tile([1, LC], F32, tag="lnistd", name="lnistd")
                nc.scalar.activation(out=istd_r, in_=lnv_r, func=AF.Exp, scale=-0.5)
                mr_r = rows.tile([1, LC], F32, tag="lnmr", name="lnmr")
                nc.vector.tensor_mul(out=mr_r, in0=mean_r, in1=istd_r)
                for j in range(NT):
                    ps_rw = ps_rep.tile([128, LC], F32, tag="rep", name="rep")
                    nc.tensor.matmul(ps_rw, lhsT=w_row[:, 128 * j : 128 * (j + 1)],
                                     rhs=istd_r)
                    ps_mrw = ps_rep.tile([128, LC], F32, tag="rep", name="rep")
                    nc.tensor.matmul(ps_mrw, lhsT=w_row[:, 128 * j : 128 * (j + 1)],
                                     rhs=mr_r)
                    nc.vector.tensor_mul(out=out_tiles[j], in0=v_tiles[j], in1=ps_rw)
                    nc.vector.scalar_tensor_tensor(
                        out=out_tiles[j], in0=out_tiles[j],
                        scalar=b_col[:, j : j + 1], in1=ps_mrw,
                        op0=OP.add, op1=OP.subtract)

            for k in range(HALF // LC):
                t0 = k * LC
                xt = [io.tile([128, LC], F32, tag=f"cx{j}", name=f"cx{j}") for j in range(NT)]
                y1 = [io.tile([128, LC], F32, tag=f"cy1{j}", name=f"cy1{j}") for j in range(NT)]
                y2 = [io.tile([128, LC], F32, tag=f"cy2{j}", name=f"cy2{j}") for j in range(NT)]
                for j in range(NT):
                    sl = slice(128 * j, 128 * (j + 1))
                    nc.sync.dma_start(out=xt[j], in_=xh[0, sl, t0 : t0 + LC])
                    nc.sync.dma_start(out=y1[j], in_=y1h[0, sl, t0 : t0 + LC])
                    nc.sync.dma_start(out=y2[j], in_=y2h[0, sl, t0 : t0 + LC])
                diff = [work.tile([128, LC], F32, tag=f"cd{j}", name=f"cd{j}") for j in range(NT)]
                for j in range(NT):
                    nc.vector.scalar_tensor_tensor(
                        out=diff[j], in0=y2[j], scalar=nl[:, 0:1], in1=y1[j],
                        op0=OP.mult, op1=OP.add)
                attn = [work.tile([128, LC], F32, tag=f"ca{j}", name=f"ca{j}") for j in range(NT)]
                layer_norm(diff, w2, b2, attn)
                res = [work.tile([128, LC], F32, tag=f"cr{j}", name=f"cr{j}") for j in range(NT)]
                for j in range(NT):
                    nc.vector.tensor_add(out=res[j], in0=xt[j], in1=attn[j])
                fin = [io.tile([128, LC], F32, tag=f"cf{j}", name=f"cf{j}") for j in range(NT)]
                layer_norm(res, w3, b3, fin)
                for j in range(NT):
                    nc.sync.dma_start(out=out[0, 128 * j : 128 * (j + 1), t0 : t0 + LC],
                                      in_=fin[j])
    return out


# ------------------------------------------------------------------- driver
def _prep_mixer_hostside(p, lnw, lnb):
    Wi = np.asarray(p["in_proj_w"], np.float32)        # (768, 384)
    W_xs, W_z = Wi[:DIM], Wi[DIM:]
    cw = np.asarray(p["conv_w"], np.float32)[:, 0, :]  # (384, 4)
    Wxs_eff = W_xs * lnw[None, :]
    Wz_eff = W_z * lnw[None, :]
    fold = np.einsum("ck,cd->kdc", cw, Wxs_eff).reshape(4 * DIM, DIM)
    bias_xs = W_xs @ lnb
    bias_total = cw.sum(1) * bias_xs + np.asarray(p["conv_b"], np.float32)
    bias_z = (W_z @ lnb)[None, :]
    bias4 = np.zeros((4, DIM), np.float32)
    bias4[0] = bias_total
    for t in range(3):
        bad = np.zeros(DIM, np.float32)
        for kk in range(DCONV):
            if t + kk - 3 < 0:
                bad += cw[:, kk] * bias_xs
        bias4[t + 1] = -bad
    return dict(
        fold_lhsT=np.ascontiguousarray(fold),
        bias4=bias4,
        wz_lhsT=np.ascontiguousarray(Wz_eff.T),
        bias_z=np.ascontiguousarray(bias_z),
        xprojT=np.ascontiguousarray(np.asarray(p["x_proj_w"], np.float32).T),
        dtprojT=np.ascontiguousarray(np.asarray(p["dt_proj_w"], np.float32).T),
        dt_bias=np.asarray(p["dt_proj_b"], np.float32)[:, None],
        d_vec=np.asarray(p["D"], np.float32)[:, None],
        outT=np.ascontiguousarray(np.asarray(p["out_proj_w"], np.float32).T),
        selbc_in=_selbc_const(),
        b4r0_in=_b4rhs_const(True),
        b4r_in=_b4rhs_const(False),
    )


def _selbc_const():
    s = np.zeros((40, 2 * N * 128), np.float32)
    for i in range(2 * N):
        s[DTR + i, 128 * i : 128 * (i + 1)] = 1.0
    return s


def _b4rhs_const(chunk0):
    r = np.zeros((4, LC), np.float32)
    r[0] = 1.0
    if chunk0:
        for t in range(3):
            r[t + 1, t] = 1.0
    return r


def kernel(x, params):
    xdt = np.asarray(x).dtype
    x = np.asarray(x, np.float32)
    lnw = np.asarray(params["norm1_w"], np.float32)
    lnb = np.asarray(params["norm1_b"], np.float32)
    m1 = _prep_mixer_hostside(params["m1"], lnw, lnb)
    m2 = _prep_mixer_hostside(params["m2"], lnw, lnb)

    xf = x.reshape(BB, DIM, L)
    names = list(m1.keys())
    stacked = {nm: np.stack([(m1 if c % 2 == 0 else m2)[nm] for c in range(8)])
               for nm in names}
    x_stack = np.stack([xf[c // 2] for c in range(8)])

    lam = 1.0 / (1.0 + np.exp(-np.float64(np.asarray(params["lambda_q"],
                                                     np.float64).sum())))
    neg_lam = np.full((8, 1, 1), -lam, np.float32)
    w2b2 = np.stack([np.stack([np.asarray(params["subln_w"], np.float32),
                               np.asarray(params["subln_b"], np.float32)])] * 8)
    w3b3 = np.stack([np.stack([np.asarray(params["norm2_w"], np.float32),
                               np.asarray(params["norm2_b"], np.float32)])] * 8)

    devs = np.asarray(jax.devices()[:8])
    mesh = Mesh(devs, ("d",))
    sh = NamedSharding(mesh, P("d"))
    put = lambda a: jax.device_put(jnp.asarray(a), sh)

    x_d = put(x_stack)
    w_d = [put(stacked[nm]) for nm in names]
    nl_d, w2_d, w3_d = put(neg_lam), put(w2b2), put(w3b3)

    @jax.jit
    def prog1(x_s, *w):
        return shard_map(_mamba_kernel, mesh=mesh,
                         in_specs=(P("d"),) * (1 + len(names)),
                         out_specs=P("d"), check_rep=False)(x_s, *w)

    @jax.jit
    def prog_exchange(x_s, y_s):
        def body(xc, yc):
            xc, yc = xc[0], yc[0]
            idx = jax.lax.axis_index("d")
            h = idx % 2
            mine = jax.lax.dynamic_slice(yc, (0, h * HALF), (DIM, HALF))
            theirs = jax.lax.dynamic_slice(yc, (0, (1 - h) * HALF), (DIM, HALF))
            perm = [(0, 1), (1, 0), (2, 3), (3, 2), (4, 5), (5, 4), (6, 7), (7, 6)]
            recv = jax.lax.ppermute(theirs, "d", perm)
            is_m1 = (idx % 2 == 0)
            y1 = jnp.where(is_m1, mine, recv)
            y2 = jnp.where(is_m1, recv, mine)
            xh = jax.lax.dynamic_slice(xc, (0, h * HALF), (DIM, HALF))
            return xh[None], y1[None], y2[None]
        return shard_map(body, mesh=mesh, in_specs=(P("d"),) * 2,
                         out_specs=(P("d"),) * 3, check_rep=False)(x_s, y_s)

    @jax.jit
    def prog_combine(xh_s, y1_s, y2_s, nl_s, w2_s, w3_s):
        return shard_map(_combine_kernel, mesh=mesh, in_specs=(P("d"),) * 6,
                         out_specs=P("d"), check_rep=False)(
                             xh_s, y1_s, y2_s, nl_s, w2_s, w3_s)

    y_d = prog1(x_d, *w_d)
    xh_d, y1_d, y2_d = prog_exchange(x_d, y_d)
    out_halves = np.asarray(prog_combine(xh_d, y1_d, y2_d, nl_d, w2_d, w3_d))

    out = np.empty((BB, DIM, L), np.float32)
    for c in range(8):
        b, h = c // 2, c % 2
        out[b, :, h * HALF : (h + 1) * HALF] = out_halves[c]
    return out.reshape(BB, DIM, 16, 14, 14).astype(xdt)


# revision 11
# speedup vs baseline: 708.0801x; 708.0801x over previous
"""DiffMamba layer on 8 Trainium2 NeuronCores.

Sharding: 8 cores = 4 samples x 2 mixers (SPMD program; per-core inputs pick
the mixer weights). Each core runs the full Mamba mixer for its (sample,
mixer) over the whole sequence, then cores exchange mixer outputs pairwise
(XLA ppermute) and each core of a pair computes the differential-combine +
layernorms for half of the sequence.

Layout: channels-on-partitions (d-major), tokens along the free dim.  The
selective scan uses the native tensor_tensor_scan (state = dA*state + dBx)
over 8 state planes, exploiting A[d,n] = -n  =>  dA_n = exp(-dt)^n.
The depthwise causal conv is folded into the in_proj matmul (K=4*384 with
token-shifted rhs slices).  norm1 scale/bias are folded into the in_proj
weights on the host.
"""

import numpy as np
import jax
import jax.numpy as jnp
from jax.sharding import Mesh, PartitionSpec as P, NamedSharding
from jax.experimental.shard_map import shard_map

import concourse.bass as bass
import concourse.tile as tile
from concourse import mybir
from concourse.bass2jax import bass_jit

F32 = mybir.dt.float32
AF = mybir.ActivationFunctionType
OP = mybir.AluOpType

DIM = 384
N = 8
DCONV = 4
DTR = 24
BB = 4
L = 3136
HALF = L // 2
LC = 392          # chunk length; 8 chunks; 4 per half
NCH = L // LC
NT = DIM // 128   # 3 channel tiles
EPS = 1e-5


def _bcast_ap(handle, parts):
    """AP reading handle[0,0,0] broadcast to (parts, 1) via partition-step 0."""
    a = handle[0, 0:1, 0:1]
    return bass.AP(tensor=a.tensor, offset=a.offset, ap=[[0, parts], [1, 1]])


# ---------------------------------------------------------------- mamba NEFF
@bass_jit
def _mamba_kernel(nc, x, fold_lhsT, bias4, wz_lhsT, bias_z, xprojT, dtprojT,
                  dt_bias, d_vec, outT, selbc_in, b4r0_in, b4r_in):
    """One (sample, mixer) Mamba pass.  x: (384, L) -> y: (384, L)."""
    y_out = nc.dram_tensor("y_out", [1, DIM, L], F32, kind="ExternalOutput")

    with tile.TileContext(nc) as tc:
        with (
            tc.tile_pool(name="persist", bufs=1) as persist,
            tc.tile_pool(name="rows", bufs=2) as rows,
            tc.tile_pool(name="io", bufs=2) as io,
            tc.tile_pool(name="work", bufs=2) as work,
            tc.tile_pool(name="scan", bufs=2) as scanp,
            tc.tile_pool(name="ps_mm", bufs=2, space="PSUM") as ps_mm,
            tc.tile_pool(name="ps_rep", bufs=2, space="PSUM") as ps_rep,
            tc.tile_pool(name="ps_row", bufs=2, space="PSUM") as ps_row,
            tc.tile_pool(name="ps_aux", bufs=2, space="PSUM") as ps_aux,
        ):
            # ---------------- persistent weights in SBUF
            w_fold = persist.tile([128, 4 * NT, DIM], F32, tag="w_fold", name="w_fold")
            nc.sync.dma_start(out=w_fold,
                              in_=fold_lhsT[0].rearrange("(a p) c -> p a c", p=128))
            w_b4 = persist.tile([4, DIM], F32, tag="w_b4", name="w_b4")
            nc.sync.dma_start(out=w_b4, in_=bias4[0])
            w_z = persist.tile([128, NT, DIM], F32, tag="w_z", name="w_z")
            nc.sync.dma_start(out=w_z, in_=wz_lhsT[0].rearrange("(a p) c -> p a c", p=128))
            w_bz = persist.tile([1, DIM], F32, tag="w_bz", name="w_bz")
            nc.sync.dma_start(out=w_bz, in_=bias_z[0])
            w_xp = persist.tile([128, NT, 40], F32, tag="w_xp", name="w_xp")
            nc.sync.dma_start(out=w_xp, in_=xprojT[0].rearrange("(a p) c -> p a c", p=128))
            w_dt = persist.tile([DTR, DIM], F32, tag="w_dt", name="w_dt")
            nc.sync.dma_start(out=w_dt, in_=dtprojT[0])
            w_out = persist.tile([128, NT, DIM], F32, tag="w_out", name="w_out")
            nc.sync.dma_start(out=w_out, in_=outT[0].rearrange("(a p) c -> p a c", p=128))
            c_dtb = persist.tile([128, NT], F32, tag="c_dtb", name="c_dtb")
            nc.sync.dma_start(out=c_dtb, in_=dt_bias[0].rearrange("(a p) o -> p (a o)", p=128))
            c_dv = persist.tile([128, NT], F32, tag="c_dv", name="c_dv")
            nc.sync.dma_start(out=c_dv, in_=d_vec[0].rearrange("(a p) o -> p (a o)", p=128))

            ones_row = persist.tile([1, LC], F32, tag="ones_row", name="ones_row")
            nc.vector.memset(ones_row, 1.0)
            eps_c = persist.tile([1, 1], F32, tag="eps_c", name="eps_c")
            nc.vector.memset(eps_c, EPS)
            ones_col = persist.tile([128, 1], F32, tag="ones_col", name="ones_col")
            nc.vector.memset(ones_col, 1.0)
            ones_1x128 = persist.tile([1, 128], F32, tag="ones_1x128", name="ones_1x128")
            nc.vector.memset(ones_1x128, 1.0)
            # row-broadcast selectors: slice i picks x_dbl row DTR+i (B rows
            # then C rows) and replicates it to all 128 partitions via K=40
            # matmul from base partition 0.  Constant pattern, host-provided.
            selbc = persist.tile([40, 2 * N * 128], F32, tag="selbc", name="selbc")
            nc.sync.dma_start(out=selbc, in_=selbc_in[0])

            # scan carry state (zeroed)
            carries = []
            for j in range(NT):
                cj = persist.tile([128, N], F32, tag=f"carry{j}", name=f"carry{j}")
                nc.vector.memset(cj, 0.0)
                carries.append(cj)

            # bias4 rhs rows: (4, LC); row0 ones; rows1-3: e_t on chunk 0
            b4_rhs0 = persist.tile([4, LC], F32, tag="b4_rhs0", name="b4_rhs0")
            nc.sync.dma_start(out=b4_rhs0, in_=b4r0_in[0])
            b4_rhs = persist.tile([4, LC], F32, tag="b4_rhs", name="b4_rhs")
            nc.sync.dma_start(out=b4_rhs, in_=b4r_in[0])

            for k in range(NCH):
                t0 = k * LC
                LH = LC + 3  # with left halo
                # ------- load x chunk with 3-col left halo
                xt = [io.tile([128, LH], F32, tag=f"xt{j}", name=f"xt{j}") for j in range(NT)]
                for j in range(NT):
                    if k == 0:
                        nc.vector.memset(xt[j][:, 0:3], 0.0)
                        nc.sync.dma_start(out=xt[j][:, 3:LH],
                                          in_=x[0, 128 * j : 128 * (j + 1), t0 : t0 + LC])
                    else:
                        nc.sync.dma_start(out=xt[j],
                                          in_=x[0, 128 * j : 128 * (j + 1), t0 - 3 : t0 + LC])

                # ------- norm1 stats over channels (PE ones-reduction)
                xsq = [work.tile([128, LH], F32, tag=f"xsq{j}", name=f"xsq{j}") for j in range(NT)]
                for j in range(NT):
                    nc.scalar.activation(out=xsq[j], in_=xt[j], func=AF.Square)
                ps_s = ps_row.tile([1, LH], F32, tag="lnrow", name="lnrow")
                ps_ss = ps_row.tile([1, LH], F32, tag="lnrow", name="lnrow")
                for j in range(NT):
                    nc.tensor.matmul(ps_s, lhsT=ones_col, rhs=xt[j],
                                     start=(j == 0), stop=(j == NT - 1))
                for j in range(NT):
                    nc.tensor.matmul(ps_ss, lhsT=ones_col, rhs=xsq[j],
                                     start=(j == 0), stop=(j == NT - 1))
                mean_r = rows.tile([1, LH], F32, tag="mean_r", name="mean_r")
                nc.scalar.mul(out=mean_r, in_=ps_s, mul=1.0 / DIM)
                ms_r = rows.tile([1, LH], F32, tag="ms_r", name="ms_r")
                nc.scalar.mul(out=ms_r, in_=ps_ss, mul=1.0 / DIM)
                var_r = rows.tile([1, LH], F32, tag="var_r", name="var_r")
                nc.vector.tensor_mul(out=var_r, in0=mean_r, in1=mean_r)
                nc.vector.tensor_sub(out=var_r, in0=ms_r, in1=var_r)
                lnv_r = rows.tile([1, LH], F32, tag="lnv_r", name="lnv_r")
                nc.scalar.activation(out=lnv_r, in_=var_r, func=AF.Ln, bias=eps_c)
                istd_r = rows.tile([1, LH], F32, tag="istd_r", name="istd_r")
                nc.scalar.activation(out=istd_r, in_=lnv_r, func=AF.Exp, scale=-0.5)
                mr_r = rows.tile([1, LH], F32, tag="mr_r", name="mr_r")
                nc.vector.tensor_mul(out=mr_r, in0=mean_r, in1=istd_r)
                # broadcast rows to 128 partitions via K=1 matmul
                ps_istd = ps_rep.tile([128, LH], F32, tag="rep", name="rep")
                nc.tensor.matmul(ps_istd, lhsT=ones_1x128, rhs=istd_r)
                ps_mr = ps_rep.tile([128, LH], F32, tag="rep", name="rep")
                nc.tensor.matmul(ps_mr, lhsT=ones_1x128, rhs=mr_r)
                # xn = x*istd - m*istd   (norm1 w/b folded into weights)
                xn = [io.tile([128, LH], F32, tag=f"xn{j}", name=f"xn{j}") for j in range(NT)]
                for j in range(NT):
                    nc.vector.tensor_mul(out=xn[j], in0=xt[j], in1=ps_istd)
                    nc.vector.tensor_sub(out=xn[j], in0=xn[j], in1=ps_mr)
                if k == 0:
                    for j in range(NT):
                        nc.vector.memset(xn[j][:, 0:3], 0.0)

                # ------- in_proj (conv folded) -> silu -> xs_act, z_silu
                xs_act = [work.tile([128, LC], F32, tag=f"xsact{j}", name=f"xsact{j}") for j in range(NT)]
                z_silu = [work.tile([128, LC], F32, tag=f"zsilu{j}", name=f"zsilu{j}") for j in range(NT)]
                brhs = b4_rhs0 if k == 0 else b4_rhs
                for mt in range(NT):
                    pxz = ps_mm.tile([128, LC], F32, tag="mm", name="mm")
                    first = True
                    for kk in range(DCONV):
                        for j in range(NT):
                            nc.tensor.matmul(
                                pxz,
                                lhsT=w_fold[:, kk * NT + j, 128 * mt : 128 * (mt + 1)],
                                rhs=xn[j][:, kk : kk + LC],
                                start=first, stop=False)
                            first = False
                    nc.tensor.matmul(pxz, lhsT=w_b4[:, 128 * mt : 128 * (mt + 1)],
                                     rhs=brhs, start=False, stop=True)
                    nc.scalar.activation(out=xs_act[mt], in_=pxz, func=AF.Silu)
                for mt in range(NT):
                    pz = ps_mm.tile([128, LC], F32, tag="mm", name="mm")
                    for j in range(NT):
                        nc.tensor.matmul(pz, lhsT=w_z[:, j, 128 * mt : 128 * (mt + 1)],
                                         rhs=xn[j][:, 3 : 3 + LC],
                                         start=(j == 0), stop=False)
                    nc.tensor.matmul(pz, lhsT=w_bz[:, 128 * mt : 128 * (mt + 1)],
                                     rhs=ones_row, start=False, stop=True)
                    nc.scalar.activation(out=z_silu[mt], in_=pz, func=AF.Silu)

                # ------- x_proj -> x_dbl (40, LC) in SBUF
                p_xd = ps_aux.tile([40, LC], F32, tag="aux", name="aux")
                for j in range(NT):
                    nc.tensor.matmul(p_xd, lhsT=w_xp[:, j, :], rhs=xs_act[j],
                                     start=(j == 0), stop=(j == NT - 1))
                xd = rows.tile([40, LC], F32, tag="xd", name="xd")
                nc.scalar.copy(out=xd, in_=p_xd)

                # ------- dt = softplus(dt_proj @ dt_raw + bias)
                # softplus(u) = ln(1 + e^u)  (no Softplus table in this build)
                dt_sb = [work.tile([128, LC], F32, tag=f"dt{j}", name=f"dt{j}") for j in range(NT)]
                for mt in range(NT):
                    pdt = ps_aux.tile([128, LC], F32, tag="aux", name="aux")
                    nc.tensor.matmul(pdt, lhsT=w_dt[:, 128 * mt : 128 * (mt + 1)],
                                     rhs=xd[0:DTR, :], start=True, stop=True)
                    eu = work.tile([128, LC], F32, tag="eu", name="eu")
                    nc.scalar.activation(out=eu, in_=pdt, func=AF.Exp,
                                         bias=c_dtb[:, mt : mt + 1])
                    nc.scalar.activation(out=dt_sb[mt], in_=eu, func=AF.Ln, bias=1.0)

                # ------- E1 = exp(-dt);  g = dt * xs
                e1 = [work.tile([128, LC], F32, tag=f"e1{j}", name=f"e1{j}") for j in range(NT)]
                g_sb = [work.tile([128, LC], F32, tag=f"g{j}", name=f"g{j}") for j in range(NT)]
                for j in range(NT):
                    nc.scalar.activation(out=e1[j], in_=dt_sb[j], func=AF.Exp, scale=-1.0)
                    nc.vector.tensor_mul(out=g_sb[j], in0=dt_sb[j], in1=xs_act[j])

                # ------- scan planes n = 1..8
                y_sb = [scanp.tile([128, LC], F32, tag=f"y{j}", name=f"y{j}") for j in range(NT)]
                ecur = [scanp.tile([128, LC], F32, tag=f"ec{j}", name=f"ec{j}") for j in range(NT)]
                for n in range(1, N + 1):
                    for j in range(NT):
                        if n == 2:
                            nc.vector.tensor_mul(out=ecur[j], in0=e1[j], in1=e1[j])
                        elif n > 2:
                            nc.vector.tensor_mul(out=ecur[j], in0=ecur[j], in1=e1[j])
                    dA = e1 if n == 1 else ecur
                    ps_bn = ps_rep.tile([128, LC], F32, tag="rep", name="rep")
                    nc.tensor.matmul(ps_bn, lhsT=selbc[:, 128 * (n - 1) : 128 * n],
                                     rhs=xd[0:40, :])
                    ps_cn = ps_rep.tile([128, LC], F32, tag="rep", name="rep")
                    nc.tensor.matmul(ps_cn, lhsT=selbc[:, 128 * (N + n - 1) : 128 * (N + n)],
                                     rhs=xd[0:40, :])
                    for j in range(NT):
                        dbx = scanp.tile([128, LC], F32, tag=f"dbx{j}", name=f"dbx{j}")
                        nc.vector.tensor_mul(out=dbx, in0=g_sb[j], in1=ps_bn)
                        h = scanp.tile([128, LC], F32, tag=f"h{j}", name=f"h{j}")
                        nc.vector.tensor_tensor_scan(
                            out=h, data0=dA[j], data1=dbx,
                            initial=carries[j][:, n - 1 : n],
                            op0=OP.mult, op1=OP.add)
                        nc.vector.tensor_copy(out=carries[j][:, n - 1 : n],
                                              in_=h[:, LC - 1 : LC])
                        if n == 1:
                            nc.vector.tensor_mul(out=y_sb[j], in0=h, in1=ps_cn)
                        else:
                            tmp = scanp.tile([128, LC], F32, tag=f"tmp{j}", name=f"tmp{j}")
                            nc.vector.tensor_mul(out=tmp, in0=h, in1=ps_cn)
                            nc.vector.tensor_add(out=y_sb[j], in0=y_sb[j], in1=tmp)

                # ------- y = (y + xs*D) * silu(z);  out_proj
                for j in range(NT):
                    nc.vector.scalar_tensor_tensor(
                        out=y_sb[j], in0=xs_act[j], scalar=c_dv[:, j : j + 1],
                        in1=y_sb[j], op0=OP.mult, op1=OP.add)
                    nc.vector.tensor_mul(out=y_sb[j], in0=y_sb[j], in1=z_silu[j])
                for mt in range(NT):
                    po = ps_mm.tile([128, LC], F32, tag="mm", name="mm")
                    for j in range(NT):
                        nc.tensor.matmul(po, lhsT=w_out[:, j, 128 * mt : 128 * (mt + 1)],
                                         rhs=y_sb[j], start=(j == 0), stop=(j == NT - 1))
                    o_sb = io.tile([128, LC], F32, tag=f"osb{mt}", name=f"osb{mt}")
                    nc.scalar.copy(out=o_sb, in_=po)
                    nc.sync.dma_start(out=y_out[0, 128 * mt : 128 * (mt + 1), t0 : t0 + LC],
                                      in_=o_sb)
    return y_out


# ------------------------------------------------------------- combine NEFF
@bass_jit
def _combine_kernel(nc, xh, y1h, y2h, neg_lam, w2b2, w3b3):
    """diff = y1 - lam*y2; attn = LN(diff)*w2+b2; out = LN(xh+attn)*w3+b3."""
    out = nc.dram_tensor("out_half", [1, DIM, HALF], F32, kind="ExternalOutput")

    with tile.TileContext(nc) as tc:
        with (
            tc.tile_pool(name="persist", bufs=1) as persist,
            tc.tile_pool(name="rows", bufs=2) as rows,
            tc.tile_pool(name="io", bufs=2) as io,
            tc.tile_pool(name="work", bufs=2) as work,
            tc.tile_pool(name="ps_row", bufs=2, space="PSUM") as ps_row,
            tc.tile_pool(name="ps_rep", bufs=2, space="PSUM") as ps_rep,
        ):
            eps_c = persist.tile([1, 1], F32, tag="eps_c", name="eps_c")
            nc.vector.memset(eps_c, EPS)
            ones_col = persist.tile([128, 1], F32, tag="ones_col", name="ones_col")
            nc.vector.memset(ones_col, 1.0)
            nl = persist.tile([128, 1], F32, tag="nl", name="nl")
            nc.sync.dma_start(out=nl, in_=_bcast_ap(neg_lam, 128))
            w2 = persist.tile([1, DIM], F32, tag="w2", name="w2")
            nc.sync.dma_start(out=w2, in_=w2b2[0, 0:1, :])
            b2 = persist.tile([128, NT], F32, tag="b2", name="b2")
            nc.sync.dma_start(out=b2,
                              in_=w2b2[0, 1:2, :].rearrange("o (a p) -> (o p) a", p=128))
            w3 = persist.tile([1, DIM], F32, tag="w3", name="w3")
            nc.sync.dma_start(out=w3, in_=w3b3[0, 0:1, :])
            b3 = persist.tile([128, NT], F32, tag="b3", name="b3")
            nc.sync.dma_start(out=b3,
                              in_=w3b3[0, 1:2, :].rearrange("o (a p) -> (o p) a", p=128))

            def layer_norm(v_tiles, w_row, b_col, out_tiles):
                vsq = [work.tile([128, LC], F32, tag=f"lnsq{j}", name=f"lnsq{j}") for j in range(NT)]
                for j in range(NT):
                    nc.scalar.activation(out=vsq[j], in_=v_tiles[j], func=AF.Square)
                ps_s = ps_row.tile([1, LC], F32, tag="lnrow", name="lnrow")
                ps_ss = ps_row.tile([1, LC], F32, tag="lnrow", name="lnrow")
                for j in range(NT):
                    nc.tensor.matmul(ps_s, lhsT=ones_col, rhs=v_tiles[j],
                                     start=(j == 0), stop=(j == NT - 1))
                for j in range(NT):
                    nc.tensor.matmul(ps_ss, lhsT=ones_col, rhs=vsq[j],
                                     start=(j == 0), stop=(j == NT - 1))
                mean_r = rows.tile([1, LC], F32, tag="lnmean", name="lnmean")
                nc.scalar.mul(out=mean_r, in_=ps_s, mul=1.0 / DIM)
                ms_r = rows.tile([1, LC], F32, tag="lnms", name="lnms")
                nc.scalar.mul(out=ms_r, in_=ps_ss, mul=1.0 / DIM)
                var_r = rows.tile([1, LC], F32, tag="lnvar", name="lnvar")
                nc.vector.tensor_mul(out=var_r, in0=mean_r, in1=mean_r)
                nc.vector.tensor_sub(out=var_r, in0=ms_r, in1=var_r)
                lnv_r = rows.tile([1, LC], F32, tag="lnlnv", name="lnlnv")
                nc.scalar.activation(out=lnv_r, in_=var_r, func=AF.Ln, bias=eps_c)
                istd_r = rows.tile([1, LC], F32, tag="lnistd", name="lnistd")
                nc.scalar.activation(out=istd_r, in_=lnv_r, func=AF.Exp, scale=-0.5)
                mr_r = rows.tile([1, LC], F32, tag="lnmr", name="lnmr")
                nc.vector.tensor_mul(out=mr_r, in0=mean_r, in1=istd_r)
                for j in range(NT):
                    ps_rw = ps_rep.tile([128, LC], F32, tag="rep", name="rep")
                    nc.tensor.matmul(ps_rw, lhsT=w_row[:, 128 * j : 128 * (j + 1)],
                                     rhs=istd_r)
                    ps_mrw = ps_rep.tile([128, LC], F32, tag="rep", name="rep")
                    nc.tensor.matmul(ps_mrw, lhsT=w_row[:, 128 * j : 128 * (j + 1)],
                                     rhs=mr_r)
                    nc.vector.tensor_mul(out=out_tiles[j], in0=v_tiles[j], in1=ps_rw)
                    nc.vector.scalar_tensor_tensor(
                        out=out_tiles[j], in0=out_tiles[j],
                        scalar=b_col[:, j : j + 1], in1=ps_mrw,
                        op0=OP.add, op1=OP.subtract)

            for k in range(HALF // LC):
                t0 = k * LC
                xt = [io.tile([128, LC], F32, tag=f"cx{j}", name=f"cx{j}") for j in range(NT)]
                y1 = [io.tile([128, LC], F32, tag=f"cy1{j}", name=f"cy1{j}") for j in range(NT)]
                y2 = [io.tile([128, LC], F32, tag=f"cy2{j}", name=f"cy2{j}") for j in range(NT)]
                for j in range(NT):
                    sl = slice(128 * j, 128 * (j + 1))
                    nc.sync.dma_start(out=xt[j], in_=xh[0, sl, t0 : t0 + LC])
                    nc.sync.dma_start(out=y1[j], in_=y1h[0, sl, t0 : t0 + LC])
                    nc.sync.dma_start(out=y2[j], in_=y2h[0, sl, t0 : t0 + LC])
                diff = [work.tile([128, LC], F32, tag=f"cd{j}", name=f"cd{j}") for j in range(NT)]
                for j in range(NT):
                    nc.vector.scalar_tensor_tensor(
                        out=diff[j], in0=y2[j], scalar=nl[:, 0:1], in1=y1[j],
                        op0=OP.mult, op1=OP.add)
                attn = [work.tile([128, LC], F32, tag=f"ca{j}", name=f"ca{j}") for j in range(NT)]
                layer_norm(diff, w2, b2, attn)
                res = [work.tile([128, LC], F32, tag=f"cr{j}", name=f"cr{j}") for j in range(NT)]
                for j in range(NT):
                    nc.vector.tensor_add(out=res[j], in0=xt[j], in1=attn[j])
                fin = [io.tile([128, LC], F32, tag=f"cf{j}", name=f"cf{j}") for j in range(NT)]
                layer_norm(res, w3, b3, fin)
                for j in range(NT):
                    nc.sync.dma_start(out=out[0, 128 * j : 128 * (j + 1), t0 : t0 + LC],
                                      in_=fin[j])
    return out


# ------------------------------------------------------------------- driver
def _prep_mixer_hostside(p, lnw, lnb):
    Wi = np.asarray(p["in_proj_w"], np.float32)        # (768, 384)
    W_xs, W_z = Wi[:DIM], Wi[DIM:]
    cw = np.asarray(p["conv_w"], np.float32)[:, 0, :]  # (384, 4)
    Wxs_eff = W_xs * lnw[None, :]
    Wz_eff = W_z * lnw[None, :]
    fold = np.einsum("ck,cd->kdc", cw, Wxs_eff).reshape(4 * DIM, DIM)
    bias_xs = W_xs @ lnb
    bias_total = cw.sum(1) * bias_xs + np.asarray(p["conv_b"], np.float32)
    bias_z = (W_z @ lnb)[None, :]
    bias4 = np.zeros((4, DIM), np.float32)
    bias4[0] = bias_total
    for t in range(3):
        bad = np.zeros(DIM, np.float32)
        for kk in range(DCONV):
            if t + kk - 3 < 0:
                bad += cw[:, kk] * bias_xs
        bias4[t + 1] = -bad
    return dict(
        fold_lhsT=np.ascontiguousarray(fold),
        bias4=bias4,
        wz_lhsT=np.ascontiguousarray(Wz_eff.T),
        bias_z=np.ascontiguousarray(bias_z),
        xprojT=np.ascontiguousarray(np.asarray(p["x_proj_w"], np.float32).T),
        dtprojT=np.ascontiguousarray(np.asarray(p["dt_proj_w"], np.float32).T),
        dt_bias=np.asarray(p["dt_proj_b"], np.float32)[:, None],
        d_vec=np.asarray(p["D"], np.float32)[:, None],
        outT=np.ascontiguousarray(np.asarray(p["out_proj_w"], np.float32).T),
        selbc_in=_selbc_const(),
        b4r0_in=_b4rhs_const(True),
        b4r_in=_b4rhs_const(False),
    )


def _selbc_const():
    s = np.zeros((40, 2 * N * 128), np.float32)
    for i in range(2 * N):
        s[DTR + i, 128 * i : 128 * (i + 1)] = 1.0
    return s


def _b4rhs_const(chunk0):
    r = np.zeros((4, LC), np.float32)
    r[0] = 1.0
    if chunk0:
        for t in range(3):
            r[t + 1, t] = 1.0
    return r


def kernel(x, params):
    xdt = np.asarray(x).dtype
    x = np.asarray(x, np.float32)
    lnw = np.asarray(params["norm1_w"], np.float32)
    lnb = np.asarray(params["norm1_b"], np.float32)
    m1 = _prep_mixer_hostside(params["m1"], lnw, lnb)
    m2 = _prep_mixer_hostside(params["m2"], lnw, lnb)

    xf = x.reshape(BB, DIM, L)
    names = list(m1.keys())
    stacked = {nm: np.stack([(m1 if c % 2 == 0 else m2)[nm] for c in range(8)])
               for nm in names}
    x_stack = np.stack([xf[c // 2] for c in range(8)])

    lam = 1.0 / (1.0 + np.exp(-np.float64(np.asarray(params["lambda_q"],
                                                     np.float64).sum())))
    neg_lam = np.full((8, 1, 1), -lam, np.float32)
    w2b2 = np.stack([np.stack([np.asarray(params["subln_w"], np.float32),
                               np.asarray(params["subln_b"], np.float32)])] * 8)
    w3b3 = np.stack([np.stack([np.asarray(params["norm2_w"], np.float32),
                               np.asarray(params["norm2_b"], np.float32)])] * 8)

    devs = np.asarray(jax.devices()[:8])
    mesh = Mesh(devs, ("d",))
    sh = NamedSharding(mesh, P("d"))
    put = lambda a: jax.device_put(jnp.asarray(a), sh)

    x_d = put(x_stack)
    w_d = [put(stacked[nm]) for nm in names]
    nl_d, w2_d, w3_d = put(neg_lam), put(w2b2), put(w3b3)

    @jax.jit
    def prog1(x_s, *w):
        return shard_map(_mamba_kernel, mesh=mesh,
                         in_specs=(P("d"),) * (1 + len(names)),
                         out_specs=P("d"), check_rep=False)(x_s, *w)

    @jax.jit
    def prog_exchange(x_s, y_s):
        def body(xc, yc):
            xc, yc = xc[0], yc[0]
            idx = jax.lax.axis_index("d")
            h = idx % 2
            mine = jax.lax.dynamic_slice(yc, (0, h * HALF), (DIM, HALF))
            theirs = jax.lax.dynamic_slice(yc, (0, (1 - h) * HALF), (DIM, HALF))
            perm = [(0, 1), (1, 0), (2, 3), (3, 2), (4, 5), (5, 4), (6, 7), (7, 6)]
            recv = jax.lax.ppermute(theirs, "d", perm)
            is_m1 = (idx % 2 == 0)
            y1 = jnp.where(is_m1, mine, recv)
            y2 = jnp.where(is_m1, recv, mine)
            xh = jax.lax.dynamic_slice(xc, (0, h * HALF), (DIM, HALF))
            return xh[None], y1[None], y2[None]
        return shard_map(body, mesh=mesh, in_specs=(P("d"),) * 2,
                         out_specs=(P("d"),) * 3, check_rep=False)(x_s, y_s)

    @jax.jit
    def prog_combine(xh_s, y1_s, y2_s, nl_s, w2_s, w3_s):
        return shard_map(_combine_kernel, mesh=mesh, in_specs=(P("d"),) * 6,
                         out_specs=P("d"), check_rep=False)(
                             xh_s, y1_s, y2_s, nl_s, w2_s, w3_s)

    y_d = prog1(x_d, *w_d)
    xh_d, y1_d, y2_d = prog_exchange(x_d, y_d)
    out_d = prog_combine(xh_d, y1_d, y2_d, nl_d, w2_d, w3_d)
    global _LAST
    _LAST = dict(prog1=prog1, prog_exchange=prog_exchange, prog_combine=prog_combine,
                 x_d=x_d, w_d=w_d, nl_d=nl_d, w2_d=w2_d, w3_d=w3_d)
    out_halves = np.asarray(out_d)

    out = np.empty((BB, DIM, L), np.float32)
    for c in range(8):
        b, h = c // 2, c % 2
        out[b, :, h * HALF : (h + 1) * HALF] = out_halves[c]
    return out.reshape(BB, DIM, 16, 14, 14).astype(xdt)


_LAST = None


def measure_device_time_ns(reps=16):
    """Estimate on-device time per full kernel invocation by pipelining
    async dispatches (amortizes the axon tunnel round-trip)."""
    import time as _t
    assert _LAST is not None, "run kernel() first"
    s = _LAST

    def run_once():
        y = s["prog1"](s["x_d"], *s["w_d"])
        xh, y1, y2 = s["prog_exchange"](s["x_d"], y)
        return s["prog_combine"](xh, y1, y2, s["nl_d"], s["w2_d"], s["w3_d"])

    run_once().block_until_ready()  # warm
    for nrep in (2, reps):
        t0 = _t.time()
        outs = [run_once() for _ in range(nrep)]
        outs[-1].block_until_ready()
        dt = _t.time() - t0
        if nrep == 2:
            t_small = dt
        else:
            t_big = dt
    per = (t_big - t_small) / (reps - 2)
    return per * 1e9
